# revision 25
# baseline (speedup 1.0000x reference)
"""Masked multi-head buffer attention on 8 TRN2 NeuronCores.

Problem shapes: x (2, 2048, 1024), buffer (2, 2048, 1024), mask (2, 2048, 2048),
Wq/Wk/Wv (1024, 1024), biases (1024,). Output (2, 2048, 1024) fp32.

Sharding: core c in 0..7 handles batch b = c//4 and head group g = c%4
(4 heads of 16). Pure data/head parallelism -- no collectives.

Schedule (per core): the ScalarE exp stream (128 ACTIVATEs of [128,1024],
~1.1us each) is the pacing engine, so everything is arranged to start it as
early as possible and never starve it:
  - x/buffer are shipped as fp8 column blocks; q/k projections run as
    fp8 DoubleRow matmuls (2 contraction chunks per MM), v as plain fp8 MMs.
    Outputs are bf16; attention math (QK^T, AV) stays bf16.
  - DMAs are issued as one serialized chain in needed-by order, so the
    critical prefix (wq, wk, x[qb0], wv, b[kb0]) lands in ~5us and the first
    exp issues at ~10us; the 8MB mask streams behind it, each k-tile chunk
    arriving just before its mask-multiply.
  - Remaining projections are dripped into the attention loop as 2-MM
    micro-chunks scheduled against static deadlines, so PE work per kt-slot
    stays under the ~1.1us exp cadence wherever possible.
  - ST[k, q] tiles (two heads packed in PE row groups 0-63/64-127) run as
    concurrent row-tiled matmul pairs; AV accumulates [dv+sum, q] in PSUM
    with a ones-column in V providing softmax row sums; AV runs LAG=2 kt
    behind exp to absorb drip jitter.
  - Per-(head, q-block) epilogue: PE-transpose to [q, dv|sum], reciprocal,
    one batched tensor_tensor scale, bf16 DMA out; the transpose scratch
    reuses the AV PSUM banks and the epilogue is spread into the next
    block's first slots so PE never stalls on it.
"""

import os
import sys

import numpy as np

for _p in ("/opt/trn_rl_repo", "/root/.axon_site/_ro/trn_rl_repo"):
    if os.path.isdir(_p) and _p not in sys.path:
        sys.path.insert(0, _p)

import ml_dtypes

B, Q, KS, D = 2, 2048, 2048, 1024
H, DK, DV = 16, 64, 64
HPC = 4  # heads per core
NCORES = 8
VW = DV + 1  # per-head v width incl. ones column
VWP = VW + 1  # padded stride for transposed epilogue tiles
QB = 512  # q block in attention phase
KT = KS // 128
SCALE = 1.0 / np.sqrt(DK)
LAG = 6  # AV runs this many kt behind exp (also covers epilogue spreading)
USE_FP8 = False  # fp8e4m3 projections measure 6.5% rel err on this problem

_GRAPH = None


def _build_graph(skip_qk_bias=True, use_fp8=USE_FP8):
    import concourse.bass as bass
    import concourse.mybir as mybir
    import concourse.tile as tile
    from concourse import bacc
    from concourse.bass import ds, ts
    from concourse.masks import make_identity
    from concourse.tile import add_dep_helper

    f32 = mybir.dt.float32
    bf16 = mybir.dt.bfloat16
    fp8 = mybir.dt.float8e4
    EXP = mybir.ActivationFunctionType.Exp
    pdt = fp8 if use_fp8 else bf16
    NQK = 8 if skip_qk_bias else 9  # contraction chunks of 128

    nc = bacc.Bacc(None)
    # x/buffer column blocks are stored partition-major so every DMA row is
    # a 2-4KB contiguous run (max descriptor efficiency): row p of block blk
    # holds [dc 0..NQK) x [512 cols] for that partition.
    xq = nc.declare_dram_parameter("xq", [4 * 128, NQK * QB], pdt, isOutput=False)
    bk = nc.declare_dram_parameter("bk", [4 * 128, NQK * QB], pdt, isOutput=False)
    wq = nc.declare_dram_parameter("wq", [128, NQK * HPC * DK], pdt, isOutput=False)
    wk = nc.declare_dram_parameter("wk", [128, NQK * HPC * DK], pdt, isOutput=False)
    wv = nc.declare_dram_parameter("wv", [128, NQK * HPC * VW], pdt, isOutput=False)
    mT = nc.declare_dram_parameter("mT", [KS, Q], bf16, isOutput=False)
    out = nc.declare_dram_parameter("out", [Q, HPC * DV], bf16, isOutput=True)

    def sub_ap(t, col, dims):
        """AP at column `col` of tile t with explicit free dims [[stride, n], ...]."""
        sl = t[:, ds(col, 1)]
        return bass.AP(tensor=sl.tensor, offset=sl.offset, ap=[sl.ap[0]] + dims)

    with tile.TileContext(nc) as tc:
        with (
            tc.tile_pool(name="consts", bufs=1) as cpool,
            tc.tile_pool(name="weights", bufs=1) as wpool,
            tc.tile_pool(name="bigin", bufs=1) as xpool,
            tc.tile_pool(name="maskp", bufs=1) as mpool,
            tc.tile_pool(name="qkv", bufs=1) as qkvpool,
            tc.tile_pool(name="ptp", bufs=10) as ptp,
            tc.tile_pool(name="epp", bufs=2) as epp,
            tc.tile_pool(name="stp", bufs=2, space="PSUM") as stp,
            tc.tile_pool(name="otp", bufs=1, space="PSUM") as otp,
            tc.tile_pool(name="pjq", bufs=1, space="PSUM") as pjq,
            tc.tile_pool(name="pjv", bufs=1, space="PSUM") as pjv,
        ):
            ident = cpool.tile([128, 128], bf16, tag="ident")
            make_identity(nc, ident[:])
            # preload the exp table set while DMAs stream
            warm = cpool.tile([128, 1], f32, tag="warm")
            nc.gpsimd.memset(warm[:], 0.0)
            nc.scalar.activation(warm[:], warm[:], EXP, scale=1.0)

            # ---------------- SBUF input tiles ----------------
            w_sb = {}
            for nm, width in (("wq", HPC * DK), ("wk", HPC * DK), ("wv", HPC * VW)):
                w_sb[nm] = wpool.tile([128, NQK * width], pdt, tag=nm, name=nm)
            x_sb = xpool.tile([128, NQK * Q], pdt, tag="x", name="x_sb")
            b_sb = xpool.tile([128, NQK * KS], pdt, tag="b", name="b_sb")
            m_sb = mpool.tile([128, KT * Q], bf16, tag="m", name="m_sb")

            qT_sb = [
                qkvpool.tile([128, Q], bf16, tag=f"qT{i}", name=f"qT{i}")
                for i in range(2)
            ]
            kT_sb = [
                qkvpool.tile([128, KS], bf16, tag=f"kT{i}", name=f"kT{i}")
                for i in range(2)
            ]
            v_sb = [
                qkvpool.tile([128, HPC * VW], bf16, tag=f"v{i}", name=f"v{i}")
                for i in range(KT)
            ]

            # ---------------- DMAs: one serialized chain, needed-by order --
            dmas = []
            wsrc = {"wq": wq, "wk": wk, "wv": wv}

            def dma_w(nm):
                d = nc.sync.dma_start(out=w_sb[nm][:], in_=wsrc[nm][:])
                dmas.append(d)

            def dma_xb(dram, sbuf, blk, half, width):
                # dc chunks [4*half, 4*half+4) of column-block blk
                nch = NQK - 4 * half if half == 1 else 4
                src = dram[
                    ds(blk * 128, 128), ds(half * 4 * QB, nch * QB)
                ].rearrange("p (c w) -> p c w", c=nch)
                dst = sub_ap(
                    sbuf, (half * 4) * width + blk * QB, [[width, nch], [1, QB]]
                )
                d = nc.sync.dma_start(out=dst, in_=src)
                dmas.append(d)

            def dma_m(kt):
                d = nc.sync.dma_start(
                    out=m_sb[:, ds(kt * Q, Q)], in_=mT[ds(kt * 128, 128), :]
                )
                dmas.append(d)

            # inputs, emitted in needed-by priority order. DMAs issue
            # round-robin across the hardware queues, so issue order gives
            # approximate priority; explicit wave-gating measures ~2us dep
            # latency per gate and is a net loss. Only the 8MB mask stream
            # is tranche-gated so it cannot starve the critical prefix.
            dma_w("wq")
            dma_w("wk")
            dma_xb(xq, x_sb, 0, 0, Q)
            dma_xb(xq, x_sb, 0, 1, Q)
            dma_w("wv")
            dma_xb(bk, b_sb, 0, 0, KS)
            dma_xb(bk, b_sb, 0, 1, KS)
            anchors = [dmas[-1]]
            dma_xb(bk, b_sb, 1, 0, KS)
            dma_xb(bk, b_sb, 1, 1, KS)
            anchors.append(dmas[-1])
            dma_xb(xq, x_sb, 1, 0, Q)
            dma_xb(bk, b_sb, 2, 0, KS)
            dma_xb(bk, b_sb, 2, 1, KS)
            anchors.append(dmas[-1])
            dma_xb(bk, b_sb, 3, 0, KS)
            dma_xb(bk, b_sb, 3, 1, KS)
            anchors.append(dmas[-1])
            dma_xb(xq, x_sb, 1, 1, Q)
            n_in = len(dmas)
            for kt in range(KT):
                dma_m(kt)
            # mask chunk kt is consumed at block-0 slot kt; under round-robin
            # the anchors themselves complete late, so release each tranche
            # one b-block earlier than its consumption window
            for kt in range(KT):
                anc = anchors[min(max(kt - 2, 0) // 5, 2)]
                add_dep_helper(
                    dmas[n_in + kt].ins, anc.ins, sync=True,
                    reason="mask DMA yields HBM bandwidth to critical inputs",
                )
            dma_xb(xq, x_sb, 2, 0, Q)
            dma_xb(xq, x_sb, 2, 1, Q)
            dma_xb(xq, x_sb, 3, 0, Q)
            dma_xb(xq, x_sb, 3, 1, Q)

            # ---------------- projection emitters ----------------
            # copies go to ScalarE early (it idles during block 0) else DVE
            def proj_copy(dst, src_ps, slot):
                if slot < 16:
                    nc.scalar.copy(dst, src_ps)
                else:
                    nc.vector.tensor_copy(dst, src_ps)

            def qk_micros(nm, hp, qc):
                """Project dst[hp][:, qc*512:+512] from w/x or w/b. 2 micro-steps."""
                sbuf = x_sb if nm == "wq" else b_sb
                dst = qT_sb[hp] if nm == "wq" else kT_sb[hp]
                wid = HPC * DK
                state = {}

                def emit_dr_pair(ps, pi):
                    lhsT = sub_ap(
                        w_sb[nm], (2 * pi) * wid + hp * 128, [[wid, 2], [1, 128]]
                    )
                    rhs = sub_ap(
                        sbuf, (2 * pi) * Q + qc * QB, [[Q, 2], [1, QB]]
                    )
                    nc.tensor.matmul(
                        ps[:], lhsT, rhs,
                        start=(pi == 0),
                        stop=(2 * pi + 2 == NQK),
                        perf_mode=mybir.MatmulPerfMode.DoubleRow,
                    )

                def emit_plain(ps, dc):
                    nc.tensor.matmul(
                        ps[:],
                        w_sb[nm][:, ds(dc * wid + hp * 128, 128)],
                        sbuf[:, ds(dc * Q + qc * QB, QB)],
                        start=(dc == 0), stop=(dc == NQK - 1),
                    )

                def mk(first, last):
                    def emit(slot):
                        if "ps" not in state:
                            state["ps"] = pjq.tile(
                                [128, QB], f32, tag="pjq", name="pjq"
                            )
                        ps = state["ps"]
                        if use_fp8:
                            if first:
                                for pi in (0, 1):
                                    emit_dr_pair(ps, pi)
                            else:
                                for pi in range(2, NQK // 2):
                                    emit_dr_pair(ps, pi)
                                if NQK % 2 == 1:
                                    emit_plain(ps, NQK - 1)
                        else:
                            dcs = (
                                range(0, NQK // 2)
                                if first
                                else range(NQK // 2, NQK)
                            )
                            for dc in dcs:
                                emit_plain(ps, dc)
                        if last:
                            proj_copy(dst[:, ts(qc, QB)], ps[:], slot)

                    return emit

                return [mk(True, False), mk(False, True)]

            def v_micros(it):
                """Project v_sb[it] (k-tile it). 4 micro-steps of 2 MMs."""
                state = {}
                starts = list(range(0, NQK, 2))

                def mk(dc0, last):
                    def emit(slot):
                        if "ps" not in state:
                            state["ps"] = pjv.tile(
                                [128, QB], f32, tag="pjv", name="pjv"
                            )
                        ps = state["ps"]
                        for dc in range(dc0, min(dc0 + 2, NQK)):
                            nc.tensor.matmul(
                                ps[:, : HPC * VW],
                                b_sb[:, ds(dc * KS + it * 128, 128)],
                                w_sb["wv"][:, ds(dc * (HPC * VW), HPC * VW)],
                                start=(dc == 0), stop=(dc == NQK - 1),
                            )
                        if last:
                            proj_copy(v_sb[it][:], ps[:, : HPC * VW], slot)
                            if skip_qk_bias:
                                nc.gpsimd.memset(v_sb[it][:, DV::VW], 1.0)

                    return emit

                return [mk(s, s == starts[-1]) for s in starts]

            # ---------------- drip schedule (static deadlines) -------------
            # item: (deadline_slot, [micro emitters])
            def av_pop_slot(it):
                # must mirror the npop schedule in the kt loop below: 1/slot
                # from kt=LAG, 2/slot from kt=10
                return LAG + it if it < 4 else 10 + (it - 4) // 2

            items = []
            for it in range(1, KT):
                items.append((av_pop_slot(it) - 1, v_micros(it)))
            for kc in range(1, 4):
                items.append((4 * kc - 2, qk_micros("wk", 0, kc)))
            for qc in range(1, 4):
                items.append((16 * qc - 2, qk_micros("wq", 0, qc)))
            for kc in range(4):
                items.append((max(61, 62 + 4 * kc - 2), qk_micros("wk", 1, kc)))
            items.append((61, qk_micros("wq", 1, 0)))
            for qc in range(1, 4):
                items.append((64 + 16 * qc - 2, qk_micros("wq", 1, qc)))
            items.sort(key=lambda x: x[0])
            dripq = []  # flattened (deadline, emit)
            for dl, micros in items:
                for m in micros:
                    dripq.append([dl, m])

            def emit_drips(slot):
                # deadline-paced: urgent items go now, otherwise at most one
                # micro per slot pulled up to 20 slots early -- front-loading
                # drips steals PE time from the exp-feeding ST/AV chain
                while dripq and dripq[0][0] <= slot + 2:
                    _, fn = dripq.pop(0)
                    fn(slot)
                if dripq and dripq[0][0] <= slot + 20:
                    _, fn = dripq.pop(0)
                    fn(slot)

            # ---------------- phase A: minimal prologue ----------------
            qA = qk_micros("wq", 0, 0)
            kA = qk_micros("wk", 0, 0)
            vA = v_micros(0)
            for m in qA:
                m(0)
            for m in kA:
                m(0)
            for m in vA:
                m(0)

            # ---------------- attention ----------------
            def do_st(hp, qlo, kt):
                st = stp.tile([128, 2 * QB], f32, tag="st", name="st")
                nc.tensor.matmul(
                    st[:, 0:QB],
                    kT_sb[hp][0:64, ts(kt, 128)],
                    qT_sb[hp][0:64, ds(qlo, QB)],
                    start=True, stop=True,
                )
                nc.tensor.matmul(
                    st[:, QB : 2 * QB],
                    kT_sb[hp][64:128, ts(kt, 128)],
                    qT_sb[hp][64:128, ds(qlo, QB)],
                    start=True, stop=True,
                )
                return st

            def emit_av(hp, kt, ptt, ot0, ot1):
                nc.tensor.matmul(
                    ot0[:VW, :],
                    v_sb[kt][:, ds((2 * hp) * VW, VW)],
                    ptt[:, 0:QB],
                    start=(kt == 0), stop=(kt == KT - 1),
                )
                nc.tensor.matmul(
                    ot1[:VW, :],
                    v_sb[kt][:, ds((2 * hp + 1) * VW, VW)],
                    ptt[:, QB : 2 * QB],
                    start=(kt == 0), stop=(kt == KT - 1),
                )

            def epilogue_parts(hp, qlo, ot0, ot1, last=False):
                """Six closures spread over following slots: copyA, copyB,
                trA, trB, scaleA, scaleB -- each PE stage waits on DVE work
                issued >=2 slots earlier so the ST chain never stalls."""
                shared = [{}, {}]
                copies, trs, scales = [], [], []
                for idx, (hh, ot_acc, tag) in enumerate(
                    ((2 * hp, ot0, "ot0"), (2 * hp + 1, ot1, "ot1"))
                ):
                    def pc(idx=idx, ot_acc=ot_acc):
                        ot_sbuf = epp.tile(
                            [128, QB], bf16, tag="otsb", name="otsb"
                        )
                        if last and idx == 0:
                            # ScalarE is idle after the final exp; halve the
                            # serial tail by splitting copies across engines
                            nc.scalar.copy(ot_sbuf[:VW, :], ot_acc[:VW, :])
                        else:
                            nc.vector.tensor_copy(ot_sbuf[:VW, :], ot_acc[:VW, :])
                        shared[idx]["osb"] = ot_sbuf

                    def pt_(idx=idx, tag=tag):
                        ot_sbuf = shared[idx]["osb"]
                        tr = otp.tile([128, 4 * VWP], bf16, tag=tag, name="tr")
                        for qt in range(4):
                            nc.tensor.transpose(
                                tr[:, ds(qt * VWP, VW)],
                                ot_sbuf[:VW, ts(qt, 128)],
                                ident[:VW, :VW],
                            )
                        rec = epp.tile([128, 4], bf16, tag="rec", name="rec")
                        with nc.allow_low_precision(
                            reason="softmax denominators are O(1e3); bf16 "
                            "reciprocal adds ~0.4% noise, within tolerance"
                        ):
                            nc.vector.reciprocal(rec[:], tr[:, DV::VWP])
                        shared[idx]["tr"] = tr
                        shared[idx]["rec"] = rec

                    def ps_(idx=idx, hh=hh, qlo=qlo):
                        tr = shared[idx]["tr"]
                        rec = shared[idx]["rec"]
                        osb = epp.tile([128, 4 * DV], bf16, tag="osb", name="osb")
                        tr_ap = bass.AP(
                            tensor=tr.tensor, offset=tr.offset,
                            ap=[tr.ap[0], [VWP, 4], [1, DV]],
                        )
                        rec_ap = bass.AP(
                            tensor=rec.tensor, offset=rec.offset,
                            ap=[rec.ap[0], [1, 4], [0, DV]],
                        )
                        nc.vector.tensor_mul(osb[:], tr_ap, rec_ap)
                        nc.sync.dma_start(
                            out=out[ds(qlo, QB), ds(hh * DV, DV)].rearrange(
                                "(qt p) w -> p qt w", p=128
                            ),
                            in_=osb[:].rearrange("p (qt w) -> p qt w", qt=4),
                        )

                    copies.append(pc)
                    trs.append(pt_)
                    scales.append(ps_)
                return [copies[0], copies[1], trs[0], trs[1], scales[0], scales[1]]

            blocks = [(hp, qb) for hp in range(2) for qb in range(Q // QB)]
            st_cur = do_st(blocks[0][0], blocks[0][1] * QB, 0)
            pending_ep = []
            for bi, (hp, qb) in enumerate(blocks):
                qlo = qb * QB
                ot0 = otp.tile([128, QB], f32, tag="ot0", name="ot0")
                ot1 = otp.tile([128, QB], f32, tag="ot1", name="ot1")
                av_pend = []
                for kt in range(KT):
                    slot = bi * KT + kt
                    if kt + 1 < KT:
                        st_next = do_st(hp, qlo, kt + 1)
                    elif bi + 1 < len(blocks):
                        nhp, nqb = blocks[bi + 1]
                        st_next = do_st(nhp, nqb * QB, 0)
                    else:
                        st_next = None
                    ptt = ptp.tile([128, 2 * QB], bf16, tag="pt", name="pt")
                    nc.scalar.activation(ptt[:], st_cur[:], EXP, scale=SCALE)
                    msl = m_sb[:, ds(kt * Q + qlo, QB)]
                    mbc = bass.AP(
                        tensor=msl.tensor, offset=msl.offset,
                        ap=[msl.ap[0], [0, 2], [1, QB]],
                    )
                    nc.vector.tensor_mul(ptt[:], ptt[:], mbc)
                    # spread previous block's epilogue into early slots
                    if pending_ep:
                        pending_ep.pop(0)()
                    emit_drips(slot)
                    av_pend.append((kt, ptt))
                    # drain 1/slot steady, 2/slot near block end so all 16
                    # land by kt15 despite the LAG-deep fill
                    npop = 2 if kt >= 10 else (1 if len(av_pend) > LAG else 0)
                    for _ in range(npop):
                        if av_pend:
                            akt, aptt = av_pend.pop(0)
                            emit_av(hp, akt, aptt, ot0, ot1)
                    st_cur = st_next
                assert not av_pend
                while pending_ep:  # drain if next block's slots didn't cover it
                    pending_ep.pop(0)()
                pending_ep = epilogue_parts(
                    hp, qlo, ot0, ot1, last=(bi == len(blocks) - 1)
                )
            while pending_ep:
                pending_ep.pop(0)()
            assert not dripq, f"{len(dripq)} drip items unscheduled"
    nc.compile()
    return nc


def _get_graph(skip_qk_bias=True):
    global _GRAPH
    key = (skip_qk_bias, USE_FP8)
    if _GRAPH is None or _GRAPH[1] != key:
        _GRAPH = (_build_graph(skip_qk_bias, USE_FP8), key)
    return _GRAPH[0]


def _prep_core_inputs(c, x, buffer, mask, Wq, bq, Wk, bk, Wv, bv):
    skip = not (np.any(bq) or np.any(bk))
    pdt = ml_dtypes.float8_e4m3 if USE_FP8 else ml_dtypes.bfloat16
    bf = ml_dtypes.bfloat16
    NQK = 8 if skip else 9
    b, g = divmod(c, 4)
    hs = slice(g * HPC * DK, (g + 1) * HPC * DK)

    xT = x[b].T  # (D, Q)
    bT = buffer[b].T  # (D, KS)

    def col_blocks(src, width):
        # (4*128, NQK*width): block blk, partition p holds its NQK d-chunks
        # contiguously so each DMA row is one long run
        out_arr = np.zeros((4, NQK, 128, width), np.float32)
        for blk in range(4):
            cols = src[:, blk * width : (blk + 1) * width]
            out_arr[blk, : D // 128] = cols.reshape(D // 128, 128, width)
            if NQK == 9:
                out_arr[blk, D // 128, 0] = 1.0
        return np.ascontiguousarray(out_arr.transpose(0, 2, 1, 3)).reshape(
            4 * 128, NQK * width
        )

    xqa = col_blocks(xT, QB)
    bka = col_blocks(bT, QB)

    def w_pmajor(wa):
        # (NQK*128, width) -> (128, NQK*width) partition-major
        width = wa.shape[1]
        return np.ascontiguousarray(
            wa.reshape(NQK, 128, width).transpose(1, 0, 2)
        ).reshape(128, NQK * width)

    wqa = np.zeros((NQK * 128, HPC * DK), np.float32)
    wqa[:D] = Wq[hs].T
    wka = np.zeros((NQK * 128, HPC * DK), np.float32)
    wka[:D] = Wk[hs].T
    wva = np.zeros((NQK * 128, HPC * VW), np.float32)
    for hh in range(HPC):
        gh = g * HPC + hh
        wva[:D, hh * VW : hh * VW + DV] = Wv[gh * DV : (gh + 1) * DV].T
    if NQK == 9:
        wqa[D] = bq[hs]
        wka[D] = bk[hs]
        for hh in range(HPC):
            gh = g * HPC + hh
            wva[D, hh * VW : hh * VW + DV] = bv[gh * DV : (gh + 1) * DV]
            wva[D, hh * VW + DV] = 1.0

    mTa = np.ascontiguousarray(mask[b].T).astype(np.float32)
    return {
        "xq": xqa.astype(pdt),
        "bk": bka.astype(pdt),
        "wq": w_pmajor(wqa).astype(pdt),
        "wk": w_pmajor(wka).astype(pdt),
        "wv": w_pmajor(wva).astype(pdt),
        "mT": mTa.astype(bf),
    }


def kernel(**inputs):
    x = np.asarray(inputs["x"], dtype=np.float32)
    buffer = np.asarray(inputs["buffer"], dtype=np.float32)
    mask = np.asarray(inputs["mask"])
    Wq = np.asarray(inputs["Wq"], dtype=np.float32)
    bq = np.asarray(inputs["bq"], dtype=np.float32)
    Wk = np.asarray(inputs["Wk"], dtype=np.float32)
    bk = np.asarray(inputs["bk"], dtype=np.float32)
    Wv = np.asarray(inputs["Wv"], dtype=np.float32)
    bv = np.asarray(inputs["bv"], dtype=np.float32)

    from concourse.bass_utils import run_bass_kernel_spmd

    skip_qk_bias = not (bq.any() or bk.any())
    nc = _get_graph(skip_qk_bias)
    in_maps = [
        _prep_core_inputs(c, x, buffer, mask, Wq, bq, Wk, bk, Wv, bv)
        for c in range(NCORES)
    ]
    res = run_bass_kernel_spmd(nc, in_maps, core_ids=list(range(NCORES)))
    full = np.empty((B, Q, H * DV), np.float32)
    for c in range(NCORES):
        b, g = divmod(c, 4)
        full[b, :, g * HPC * DV : (g + 1) * HPC * DV] = res.results[c]["out"]
    return full
